# revision 17
# baseline (speedup 1.0000x reference)
"""Trainium2 Bass kernel for nn_AffineTransformer_6442450944616.

kernel(**inputs): FULL inputs -> (fill_out, stroke_out) [2048,128,128] f32,
matching reference.reference().  Data-parallel over samples, 8 cores.

Wall time under axon is dominated by host<->device transfer (~55-80 MB/s,
half-duplex tunnel), so the kernel minimizes transferred bytes and
overlaps everything it can:
  - images are sent as uint8 (x255), 16.8MB instead of 67MB f32
  - the output is SPARSE-COMPACTED on device: only quarter-rows (32px)
    whose bilinear-warp support is nonempty are downloaded.  The support
    is an interval per output row, computed exactly on the host from the
    affine params; kept quarter-row slots get sequential slot indices
    (uint16) and the device scatters them into a compact [XSLOTS,32]
    uint8 tensor with indirect DMAs (out-of-range index = dropped slot).
    ~25% of slots are kept -> ~19MB download instead of 268MB f32 dense.
    If a core's kept slots exceed the static capacity, the overflow
    slots are computed on the host (exact, vectorized) — a rarely-taken
    safety net for input-distribution shift.
  - output values are uint8 (x255); end-to-end quantization rel err
    ~3.5e-3 vs tolerance 2e-2
  - the batch runs as TWO half-batch device calls (ns=128/core each) so
    device exec of one half hides under the tunnel stream of the other
  - affine params are sent as [ns,8] f32 and expanded on device; pj/qj
    pixel grids are generated on-device with iota; output donation
    buffers are created on-device (run_bass_kernel_spmd would upload
    them as host zeros every call) via a runner modeled on
    bass2jax.run_bass_via_pjrt with a cached jitted callable
  - image upload is dispatched per core-shard as soon as each shard is
    quantized; fetch+dequant+reconstruct run per shard so host work
    overlaps the tunnel stream

Math per sample i, pixel j (p=j//128, q=j%128):
  ix(j)=t00*q+t01*p+Cx ; iy likewise
  out[j] = sum_{x,y payload} relu(1-|ix-x|) * relu(1-|iy-y|) * img[y,x]
(exact bilinear-with-zeros; hat weights equal (1-w, w) on live taps).
A pixel can be nonzero only if ix in (-1,64) and iy in (-1,64); for
fixed p both are linear in q, so the support is a q-interval per row ->
the host knows exactly which quarter-rows matter (eps-margined for f32).
"""
import numpy as np
import jax
import jax.numpy as jnp
from jax.sharding import Mesh, NamedSharding, PartitionSpec
from jax.experimental.shard_map import shard_map

import concourse.bass as bass
import concourse.bacc as bacc
import concourse.tile as tile
import concourse.mybir as mybir
from concourse import bass2jax

F32 = mybir.dt.float32
BF16 = mybir.dt.bfloat16
I32 = mybir.dt.int32
U8 = mybir.dt.uint8
U16 = mybir.dt.uint16
AL = mybir.AluOpType
ACTF = mybir.ActivationFunctionType

N = 2048
NCORES = 8
NHALF = 4               # split the batch into 4 pipelined device calls
NS = N // NCORES // NHALF   # 64 samples per core per call
P = 128
NPIX = P * P
CH = 1024
NCH = NPIX // CH
XSLOTS = 18944          # compact quarter-row slot capacity per core-call
EPS = 0.05              # support-interval widening (covers f32 rounding)


def _build(ns: int):
    nc = bacc.Bacc("TRN2", target_bir_lowering=False, debug=False)
    ibt_d = nc.dram_tensor("ibt", [ns, 64, P], U8, kind="ExternalInput")
    wc6_d = nc.dram_tensor("wc6", [ns, 8], F32, kind="ExternalInput")
    sidx_d = nc.dram_tensor("sidx", [ns, P, 8], U16, kind="ExternalInput")
    comp_d = nc.dram_tensor("comp", [XSLOTS, 32], U8, kind="ExternalOutput")

    with tile.TileContext(nc) as tc:
        with tc.tile_pool(name="const", bufs=1) as cpool, \
             tc.tile_pool(name="work", bufs=3) as pool, \
             tc.tile_pool(name="out", bufs=2) as opool, \
             tc.tile_pool(name="ps", bufs=2, space="PSUM") as psum, \
             tc.tile_pool(name="psw", bufs=1, space="PSUM") as psumw:
            # on-device constants: local pixel grids, chunk offsets,
            # per-partition p%64, matmul helper matrices
            pj0i = cpool.tile([P, CH], I32, tag="pj0i")
            qj0i = cpool.tile([P, CH], I32, tag="qj0i")
            c8i = cpool.tile([P, NCH], I32, tag="c8i")
            pm64i = cpool.tile([P, 1], I32, tag="pm64i")
            nc.gpsimd.iota(pj0i[:], pattern=[[1, 8], [0, P]], base=0,
                           channel_multiplier=0)
            nc.gpsimd.iota(qj0i[:], pattern=[[0, 8], [1, P]], base=0,
                           channel_multiplier=0)
            nc.gpsimd.iota(c8i[:], pattern=[[8, NCH]], base=0,
                           channel_multiplier=0)
            nc.gpsimd.iota(pm64i[0:64, :], pattern=[[0, 1]], base=0,
                           channel_multiplier=1)
            nc.gpsimd.iota(pm64i[64:128, :], pattern=[[0, 1]], base=0,
                           channel_multiplier=1)
            pj0 = cpool.tile([P, CH], F32, tag="pj0")
            qj0 = cpool.tile([P, CH], F32, tag="qj0")
            c8 = cpool.tile([P, NCH], F32, tag="c8")
            pm64 = cpool.tile([P, 1], F32, tag="pm64")
            nc.scalar.copy(out=pj0[:], in_=pj0i[:])
            nc.scalar.copy(out=qj0[:], in_=qj0i[:])
            nc.scalar.copy(out=c8[:], in_=c8i[:])
            nc.scalar.copy(out=pm64[:], in_=pm64i[:])
            ones2 = cpool.tile([P, 2], BF16, tag="ones2")
            nc.vector.memset(ones2[:], 0.0)
            nc.vector.memset(ones2[0:64, 0:1], 1.0)
            nc.vector.memset(ones2[64:128, 1:2], 1.0)
            one1 = cpool.tile([1, P], F32, tag="one1")
            nc.vector.memset(one1[:], 1.0)

            with tc.For_i(0, ns, 1) as i:
                w6 = pool.tile([1, 8], F32, tag="w6", name=f"w6{i}")
                ibtu = pool.tile([64, P], U8, tag="ibtu", name=f"ibtu{i}")
                stile = pool.tile([P, 8], U16, tag="stile", name=f"st{i}")
                nc.sync.dma_start(out=w6[:], in_=wc6_d[bass.ds(i, 1), :])
                nc.sync.dma_start(out=ibtu[:], in_=ibt_d[bass.ds(i, 1), :, :])
                nc.sync.dma_start(out=stile[:], in_=sidx_d[bass.ds(i, 1), :, :])
                stile32 = pool.tile([P, 8], I32, tag="stile32", name=f"s32{i}")
                nc.scalar.copy(out=stile32[:], in_=stile[:])
                ibtf = pool.tile([64, P], BF16, tag="ibtf", name=f"ibtf{i}")
                nc.scalar.copy(out=ibtf[:], in_=ibtu[:])
                # broadcast w6 row to all partitions, then select per-half
                wcb = psumw.tile([P, 8], F32, tag="wcb", name=f"wcb{i}")
                nc.tensor.matmul(out=wcb[:], lhsT=one1[:], rhs=w6[:],
                                 start=True, stop=True)
                wcs = pool.tile([P, 8], F32, tag="wcs", name=f"wcs{i}")
                nc.scalar.copy(out=wcs[:], in_=wcb[:])
                wcf0 = pool.tile([P, 1], F32, tag="wcf0", name=f"wcf0{i}")
                nc.scalar.copy(out=wcf0[0:64, :], in_=wcs[0:64, 0:1])
                nc.scalar.copy(out=wcf0[64:128, :], in_=wcs[64:128, 3:4])
                wcf1 = pool.tile([P, 1], F32, tag="wcf1", name=f"wcf1{i}")
                nc.scalar.copy(out=wcf1[0:64, :], in_=wcs[0:64, 1:2])
                nc.scalar.copy(out=wcf1[64:128, :], in_=wcs[64:128, 4:5])
                wc2f = pool.tile([P, 1], F32, tag="wc2f", name=f"wc2f{i}")
                nc.scalar.activation(out=wc2f[0:64, :], in_=pm64[0:64, :],
                                     func=ACTF.Identity, scale=-1.0,
                                     bias=wcs[0:64, 2:3])
                nc.scalar.activation(out=wc2f[64:128, :], in_=pm64[64:128, :],
                                     func=ACTF.Identity, scale=-1.0,
                                     bias=wcs[64:128, 5:6])
                # per-chunk abs-bias: wc2f + 8c*wcf0
                wc2c = pool.tile([P, NCH], F32, tag="wc2c", name=f"wc2c{i}")
                nc.scalar.activation(out=wc2c[:], in_=c8[:], func=ACTF.Identity,
                                     scale=wcf0[:], bias=wc2f[:])
                o2 = opool.tile([2, NPIX], U8, tag="o2", name=f"o2_{i}")
                for c in range(NCH):
                    d1 = pool.tile([P, CH], F32, tag="d1", name=f"d1_{c}")
                    nc.gpsimd.tensor_scalar(d1[:], pj0[:], wcf0[:], None,
                                            AL.mult)
                    d2 = pool.tile([P, CH], F32, tag="d2", name=f"d2_{c}")
                    nc.vector.scalar_tensor_tensor(d2[:], qj0[:], wcf1[:], d1[:],
                                                   AL.mult, AL.add)
                    ab = pool.tile([P, CH], F32, tag="ab", name=f"ab_{c}")
                    nc.scalar.activation(out=ab[:], in_=d2[:], func=ACTF.Abs,
                                         scale=1.0, bias=wc2c[:, c:c + 1])
                    hh = pool.tile([P, CH], BF16, tag="hh", name=f"hh_{c}")
                    nc.scalar.activation(out=hh[:], in_=ab[:], func=ACTF.Relu,
                                         scale=-1.0, bias=1.0)
                    cc = psum.tile([P, CH], F32, tag="C", name=f"cc_{c}")
                    for h in range(CH // 512):
                        hs = slice(h * 512, (h + 1) * 512)
                        nc.tensor.matmul(out=cc[:, hs], lhsT=ibtf[:], rhs=hh[0:64, hs],
                                         start=True, stop=True)
                    mm = pool.tile([P, CH], BF16, tag="mm", name=f"mm_{c}")
                    nc.vector.tensor_tensor(mm[0:64, :], cc[0:64, :], hh[64:128, :],
                                            AL.mult)
                    nc.vector.tensor_tensor(mm[64:128, :], cc[64:128, :],
                                            hh[64:128, :], AL.mult)
                    for h in range(CH // 512):
                        hs = slice(h * 512, (h + 1) * 512)
                        oo = psum.tile([2, 512], F32, tag="O", name=f"oo_{c}_{h}")
                        nc.tensor.matmul(out=oo[:], lhsT=ones2[:], rhs=mm[:, hs],
                                         start=True, stop=True)
                        # f32 PSUM -> uint8 SBUF (round-to-nearest, saturating)
                        nc.scalar.activation(out=o2[:, c * CH + h * 512:
                                                    c * CH + (h + 1) * 512],
                                             in_=oo[:], func=ACTF.Copy, scale=1.0)
                # reshape pixels row-major onto partitions:
                # o2r[p, 128*m + q] = map m, row p, col q
                o2r = opool.tile([P, 256], U8, tag="o2r", name=f"o2r_{i}")
                nc.sync.dma_start(out=o2r[:, 0:128], in_=o2[0:1, :])
                nc.sync.dma_start(out=o2r[:, 128:256], in_=o2[1:2, :])
                # scatter kept quarter-rows to their compact slots; index
                # XSLOTS (> bounds) drops the slot.  g = 4*m + qh.
                for g in range(8):
                    nc.gpsimd.indirect_dma_start(
                        out=comp_d[:, :],
                        out_offset=bass.IndirectOffsetOnAxis(
                            ap=stile32[:, g:g + 1], axis=0),
                        in_=o2r[:, 32 * g:32 * (g + 1)],
                        in_offset=None,
                        bounds_check=XSLOTS - 1,
                        oob_is_err=False)
    nc.compile()
    return nc


class _Runtime:
    pass


_CACHE = {}


def _get_runtime() -> _Runtime:
    if "rt" in _CACHE:
        return _CACHE["rt"]
    nc = _build(NS)
    bass2jax.install_neuronx_cc_hook()
    assert nc.dbg_addr is None

    in_names, out_names, out_avals = [], [], []
    partition_name = (nc.partition_id_tensor.name
                      if nc.partition_id_tensor else None)
    for alloc in nc.m.functions[0].allocations:
        if not isinstance(alloc, mybir.MemoryLocationSet):
            continue
        name = alloc.memorylocations[0].name
        if alloc.kind == "ExternalInput":
            if name != partition_name:
                in_names.append(name)
        elif alloc.kind == "ExternalOutput":
            out_names.append(name)
            out_avals.append(jax.core.ShapedArray(
                tuple(alloc.tensor_shape), mybir.dt.np(alloc.dtype)))
    n_params = len(in_names)
    n_outs = len(out_names)
    full_in_names = list(in_names) + list(out_names)
    if partition_name is not None:
        full_in_names.append(partition_name)

    def _body(*args):
        operands = list(args)
        if partition_name is not None:
            operands.append(bass2jax.partition_id_tensor())
        outs = bass2jax._bass_exec_p.bind(
            *operands,
            out_avals=tuple(out_avals),
            in_names=tuple(full_in_names),
            out_names=tuple(out_names),
            lowering_input_output_aliases=(),
            sim_require_finite=True,
            sim_require_nnan=True,
            nc=nc,
        )
        return tuple(outs)

    mesh = Mesh(np.asarray(jax.devices()[:NCORES]), ("core",))
    donate = tuple(range(n_params, n_params + n_outs))
    sharded = jax.jit(
        shard_map(_body, mesh=mesh,
                  in_specs=(PartitionSpec("core"),) * (n_params + n_outs),
                  out_specs=(PartitionSpec("core"),) * n_outs,
                  check_rep=False),
        donate_argnums=donate, keep_unused=True)
    sh = NamedSharding(mesh, PartitionSpec("core"))
    zshapes = [(NCORES * a.shape[0], *a.shape[1:]) for a in out_avals]
    zdtypes = [a.dtype for a in out_avals]
    zeros_fn = jax.jit(
        lambda: tuple(jnp.zeros(s, d) for s, d in zip(zshapes, zdtypes)),
        out_shardings=(sh,) * n_outs)

    rt = _Runtime()
    rt.in_names = in_names
    rt.out_names = out_names
    rt.sharded = sharded
    rt.zeros_fn = zeros_fn
    rt.sh = sh
    rt.devices = list(jax.devices()[:NCORES])
    _CACHE["rt"] = rt
    return rt


def _theta_host(affine_outs):
    a = affine_outs.astype(np.float64)
    sig = lambda v: 1.0 / (1.0 + np.exp(-v))
    t00 = 2 * sig(a[:, 0]); t11 = 2 * sig(a[:, 1])
    t01 = 2 * np.tanh(a[:, 2]); t10 = 2 * np.tanh(a[:, 3])
    t02 = np.tanh(a[:, 4]); t12 = np.tanh(a[:, 5])
    cx = (t00 + t01) * (0.5 - 64.0) + 64.0 * t02 + 63.5
    cy = (t10 + t11) * (0.5 - 64.0) + 64.0 * t12 + 63.5
    return t00, t01, t10, t11, cx - 32.0, cy - 32.0


def _keep_table(t00, t01, t10, t11, cxp, cyp):
    """keep[i,p,qh]: quarter-row (p, q in [32qh,32qh+32)) support test."""
    p = np.arange(128.0)
    b1 = t01[:, None] * p + cxp[:, None]
    ql1 = (-1.0 - EPS - b1) / t00[:, None]
    qh1 = (64.0 + EPS - b1) / t00[:, None]
    b2 = t11[:, None] * p + cyp[:, None]
    s = t10[:, None]
    with np.errstate(divide="ignore", invalid="ignore"):
        a2 = (-1.0 - EPS - b2) / s
        b2b = (64.0 + EPS - b2) / s
    ql2 = np.minimum(a2, b2b); qh2 = np.maximum(a2, b2b)
    tiny = np.abs(s) < 1e-12
    inr = (b2 > -1.0 - EPS) & (b2 < 64.0 + EPS)
    ql2 = np.where(tiny, np.where(inr, -1e9, 1e9), ql2)
    qh2 = np.where(tiny, np.where(inr, 1e9, -1e9), qh2)
    ql = np.maximum(ql1, ql2); qh = np.minimum(qh1, qh2)
    return np.stack([(qh >= 32.0 * h) & (ql <= 32.0 * h + 31.0)
                     for h in range(4)], axis=-1)


def _host_slots(slots, t00, t01, t10, t11, cxp, cyp, fq, sq):
    """Exact uint8-pipeline values for (ilocal, m, p, qh) overflow slots.

    fq, sq: quantized images (uint8 values as float) for the slots'
    sample set, [k?,64,64] indexed by slots[:,0].  Returns [k,32] float32
    (already /255)."""
    ii, mm_, pp, hh_ = slots.T
    qs = hh_[:, None] * 32.0 + np.arange(32.0)[None, :]      # [k,32]
    ix = t00[ii][:, None] * qs + (t01[ii] * pp + cxp[ii])[:, None]
    iy = t10[ii][:, None] * qs + (t11[ii] * pp + cyp[ii])[:, None]
    img = np.where(mm_[:, None, None] == 0, fq[ii], sq[ii])  # [k,64,64]
    x0 = np.floor(ix); y0 = np.floor(iy)
    wx = ix - x0; wy = iy - y0
    acc = np.zeros_like(ix)
    for dy in (0, 1):
        for dx in (0, 1):
            xf = x0 + dx; yf = y0 + dy
            w = (wx if dx else 1 - wx) * (wy if dy else 1 - wy)
            valid = (xf >= 0) & (xf <= 63) & (yf >= 0) & (yf <= 63)
            xi = np.clip(xf, 0, 63).astype(np.int64)
            yi = np.clip(yf, 0, 63).astype(np.int64)
            v = np.take_along_axis(
                img.reshape(img.shape[0], -1),
                (yi * 64 + xi).reshape(img.shape[0], -1), axis=1
            ).reshape(ix.shape)
            acc += np.where(valid, v, 0.0) * w
    return (np.rint(acc) * (1.0 / 255.0)).astype(np.float32)


def kernel(affine_outs, fill_alpha, stroke_alpha, targetsize):
    affine_outs = np.asarray(affine_outs, dtype=np.float32)
    fill_alpha = np.asarray(fill_alpha)
    stroke_alpha = np.asarray(stroke_alpha)
    rt = _get_runtime()
    devs = rt.devices
    half = np.float32(0.5)
    s255 = np.float32(255.0)
    t00, t01, t10, t11, cxp, cyp = _theta_host(affine_outs)
    wc6 = np.zeros((N, 8), np.float32)
    wc6[:, 0] = t01; wc6[:, 1] = t00; wc6[:, 2] = cxp
    wc6[:, 3] = t11; wc6[:, 4] = t10; wc6[:, 5] = cyp
    keep = None

    fill_out = np.zeros((N, P, P), np.float32)
    stroke_out = np.zeros((N, P, P), np.float32)
    inv = np.float32(1.0 / 255.0)
    m_of = np.empty((NS, 2, P, 4), np.int8)
    m_of[:, 0] = 0; m_of[:, 1] = 1

    # global sample index ranges: slice h, core c
    def hslice(h, c):
        return slice(c * NHALF * NS + h * NS, c * NHALF * NS + (h + 1) * NS)

    halves = []
    for h in range(NHALF):
        # quantize/pack per core-shard, dispatch each upload immediately
        ibt_shards, fqs, sqs = [], [], []
        for c in range(NCORES):
            sl = hslice(h, c)
            fq = (fill_alpha[sl] * s255 + half).astype(np.uint8)
            sq = (stroke_alpha[sl] * s255 + half).astype(np.uint8)
            fqs.append(fq); sqs.append(sq)
            ibt_c = np.empty((NS, 64, P), np.uint8)
            ibt_c[:, :, :64] = fq.transpose(0, 2, 1)
            ibt_c[:, :, 64:] = sq.transpose(0, 2, 1)
            ibt_shards.append(jax.device_put(ibt_c, devs[c]))
        d_ibt = jax.make_array_from_single_device_arrays(
            (NCORES * NS, 64, P), rt.sh, ibt_shards)
        if keep is None:
            keep = _keep_table(t00, t01, t10, t11, cxp, cyp)  # [N,128,4]
        gidx = np.concatenate([np.arange(hslice(h, c).start,
                                         hslice(h, c).stop)
                               for c in range(NCORES)])
        wc6_h = wc6[gidx]
        keep4 = np.repeat(keep[gidx].reshape(
            NCORES, NS, P, 4)[:, :, None], 2, axis=2)  # [8,ns,2,128,4]
        sidx = np.empty((NCORES * NS, P, 8), np.uint16)
        core_info = []
        for c in range(NCORES):
            kc = keep4[c]                                  # [ns,2,128,4]
            flat = kc.reshape(-1)
            idx = np.cumsum(flat, dtype=np.int64) - 1
            idx = np.where(flat, idx, np.int64(XSLOTS))
            over = idx >= XSLOTS
            idx = np.where(over, np.int64(XSLOTS), idx)
            # sidx[i,p,4m+qh] = idx[(i,m,p,qh)]
            sidx[c * NS:(c + 1) * NS] = idx.reshape(
                NS, 2, P, 4).transpose(0, 2, 1, 3).reshape(NS, P, 8)
            core_info.append((kc, flat & ~over.reshape(flat.shape)))
        ins = {"ibt": d_ibt, "wc6": wc6_h, "sidx": sidx}
        outs = rt.sharded(*[ins[name] for name in rt.in_names],
                          *rt.zeros_fn())
        arr = dict(zip(rt.out_names, outs))["comp"]
        shards = sorted(arr.addressable_shards,
                        key=lambda s: s.index[0].start or 0)
        for s in shards:
            s.data.copy_to_host_async()
        halves.append((shards, core_info, fqs, sqs))

    for h, (shards, core_info, fqs, sqs) in enumerate(halves):
        for cshard in shards:
            c = (cshard.index[0].start or 0) // XSLOTS
            kc, eff_flat = core_info[c]
            eff = eff_flat.reshape(kc.shape)
            nk = int(eff_flat.sum())
            buf = np.asarray(cshard.data)                  # [XSLOTS,32] u8
            vals = np.multiply(buf[:nk], inv, dtype=np.float32)
            sm = m_of[eff]                                 # [nk] map ids
            g0 = hslice(h, c)
            fv = fill_out[g0].reshape(NS, P, 4, 32)
            sv = stroke_out[g0].reshape(NS, P, 4, 32)
            fv[eff[:, 0]] = vals[sm == 0]
            sv[eff[:, 1]] = vals[sm == 1]
            # overflow slots (idx beyond capacity): compute on host (rare)
            dropped = kc & ~eff
            if dropped.any():
                slots = np.argwhere(dropped)
                hv = _host_slots(slots, t00[g0], t01[g0], t10[g0],
                                 t11[g0], cxp[g0], cyp[g0],
                                 fqs[c].astype(np.float64),
                                 sqs[c].astype(np.float64))
                smv = slots[:, 1]
                fv[dropped[:, 0]] = hv[smv == 0]
                sv[dropped[:, 1]] = hv[smv == 1]
    return fill_out, stroke_out